# revision 2
# baseline (speedup 1.0000x reference)
"""Llama-style 2-layer transformer forward, tensor-parallel + sequence-parallel
on 8 NeuronCores (Megatron-SP style), with pipelined half-AllGathers.

TP: q/k/v/gate/up column-sharded, o/down row-sharded; core c owns q heads
{2c, 2c+1}, kv head c, I-slice [c*1024,(c+1)*1024).
SP: the residual x lives sequence-sharded (core c holds its 256 tokens) in
H-major layout [128 H-partitions, 16 H-chunks, 256 tokens] bf16.

Boundary flow: the RAW residual shard is AllGathered in two 128-token halves
(so downstream compute starts as soon as the first half lands), each core
redundantly rmsnorms the gathered chunks in place, runs the column-parallel
matmuls, and the row-parallel o/down partial sums are ReduceScattered
(8MB -> 1MB) back onto the local shard.  RS+AG moves the same bytes as an
AllReduce but the RS output is 8x smaller and nothing sits serially behind
the gather except the RS itself.

All matmuls are transpose-free via the H-major layout: ones-matmul rmsnorm
statistics, a PE rotation matrix for rope, transposed scores with a
ones-column in V for the softmax denominator, weight-stationary row-parallel
projections.  Gathered-token chunks are (half j, rank-group g) sets of
4 x 128 strided tokens; tensors indexed [rank, half, 128] remain in global
token order because rank-major x half x token equals the global ordering.
"""

import numpy as np
import ml_dtypes

import concourse.bass as bass
import concourse.tile as tile
from concourse import bacc, mybir
from concourse import bass_utils
from concourse.bass import ds
from concourse.masks import make_identity

P = 128
B, S, H, NH, NKV, L, I, V = 1, 2048, 2048, 16, 8, 2, 8192, 32000
HD = H // NH            # 128
NCORES = 8
QH = NH // NCORES       # 2 q heads per core
IC = I // NCORES        # 1024 intermediate cols per core
ODC = QH                # o-proj contraction chunks of 128
IB = IC // P            # 8 I-blocks per core
KS = H // P             # 16 contraction subtiles over H
TCC = S // NCORES       # 256 own tokens
HB = TCC // 2           # 128-token AllGather half-blocks
TOK = 512               # token chunk for projections / MLP
NB = TOK // HB          # 4 rank-subblocks per compute chunk
NQC = S // TOK          # 4
TOKA = 256              # token chunk for attention scores
NQA = S // TOKA         # 8
NKC = S // P            # 16 key chunks
WDC = 256               # wd H-col streaming chunk
NHG = H // WDC          # 8 H-col groups for wd streaming
EPS = 1e-5
THETA = 10000.0
SCALE = HD ** -0.5

BF = mybir.dt.bfloat16
F32 = mybir.dt.float32
AF = mybir.ActivationFunctionType
OP = mybir.AluOpType

LAST_RESULT = None
LAST_NC = None
LAST_IN_MAPS = None


def _build():
    nc = bacc.Bacc("TRN2", target_bir_lowering=False, debug=False,
                   enable_asserts=False, num_devices=NCORES)

    x0_ap = nc.dram_tensor("x0", [P, KS, TCC], BF, kind="ExternalInput").ap()
    wq_ap = nc.dram_tensor("wq", [L, P, KS, QH * HD], BF, kind="ExternalInput").ap()
    wk_ap = nc.dram_tensor("wk", [L, P, KS, HD], BF, kind="ExternalInput").ap()
    wv_ap = nc.dram_tensor("wv", [L, P, KS, HD], BF, kind="ExternalInput").ap()
    wo_ap = nc.dram_tensor("wo", [L, P, ODC, H], BF, kind="ExternalInput").ap()
    wg_ap = nc.dram_tensor("wg", [L, P, IB, KS, HD], BF, kind="ExternalInput").ap()
    wu_ap = nc.dram_tensor("wu", [L, P, IB, KS, HD], BF, kind="ExternalInput").ap()
    wd_ap = nc.dram_tensor("wd", [L, P, NHG, IB, WDC], BF, kind="ExternalInput").ap()
    cos_ap = nc.dram_tensor("cosT", [P, S], BF, kind="ExternalInput").ap()
    sin_ap = nc.dram_tensor("sinT", [P, S], BF, kind="ExternalInput").ap()
    rt_ap = nc.dram_tensor("rotT", [P, P], BF, kind="ExternalInput").ap()
    nw_ap = nc.dram_tensor("nwT", [P, KS], F32, kind="ExternalInput").ap()
    out_ap = nc.dram_tensor("out", [P, KS, TCC], F32, kind="ExternalOutput").ap()

    RG = [list(range(NCORES))]

    with tile.TileContext(nc) as tc:
        with (
            tc.tile_pool(name="const", bufs=1) as const,
            tc.tile_pool(name="own", bufs=1) as own,
            tc.tile_pool(name="hch", bufs=2) as hpool,
            tc.tile_pool(name="retp", bufs=2) as retp,
            tc.tile_pool(name="qkv", bufs=1) as qkv,
            tc.tile_pool(name="attp", bufs=2) as attp,
            tc.tile_pool(name="actp", bufs=2) as actp,
            tc.tile_pool(name="wbig", bufs=1) as wbig,
            tc.tile_pool(name="wstr", bufs=2) as wstr,
            tc.tile_pool(name="scr", bufs=2) as scr,
            tc.tile_pool(name="ps_a", bufs=3, space="PSUM") as ps_a,
            tc.tile_pool(name="ps_b", bufs=3, space="PSUM") as ps_b,
            tc.tile_pool(name="ps_c", bufs=2, space="PSUM") as ps_c,
            tc.tile_pool(name="dram", bufs=1, space="DRAM") as dram,
        ):
            ident = const.tile([P, P], BF)
            make_identity(nc, ident[:])
            # cos/sin laid out [P, rank, half, 128] == [P, S] global order
            cos_sb = const.tile([P, NCORES, 2, HB], BF)
            nc.sync.dma_start(cos_sb[:], cos_ap[:])
            sin_sb = const.tile([P, NCORES, 2, HB], BF)
            nc.sync.dma_start(sin_sb[:], sin_ap[:])
            rt_sb = const.tile([P, P], BF)
            nc.sync.dma_start(rt_sb[:], rt_ap[:])
            nw_sb = const.tile([P, KS], F32)
            nc.sync.dma_start(nw_sb[:], nw_ap[:])
            ones_h = const.tile([P, 1], BF)
            nc.vector.memset(ones_h[:], 1.0)
            ones_r = const.tile([1, P], F32)
            nc.vector.memset(ones_r[:], 1.0)

            x_own = own.tile([P, KS, TCC], BF)
            nc.sync.dma_start(x_own[:], x0_ap[:])

            def gather_x(tagn):
                """AllGather x_own in two 128-token halves -> per-half
                [NCORES, P, KS, HB] shared DRAM tiles."""
                agx_in = dram.tile([2, P, KS, HB], BF, tag=f"agi_{tagn}",
                                   name=f"agi_{tagn}")
                outs = []
                for j in range(2):
                    nc.sync.dma_start(agx_in[j], x_own[:, :, ds(j * HB, HB)])
                    ago = dram.tile([NCORES, P, KS, HB], BF,
                                    tag=f"ago_{tagn}_{j}", name=f"ago_{tagn}_{j}",
                                    addr_space="Shared")
                    nc.gpsimd.collective_compute(
                        "AllGather", OP.bypass, replica_groups=RG,
                        ins=[agx_in[j].opt()], outs=[ago.opt()],
                    )
                    outs.append(ago)
                return outs

            def rs_and_add(ar_in, tagn):
                """ReduceScatter staged partials, add own block to x_own.
                ar_in layout [NCORES, 2, P, KS, HB] (rank-major, half-minor)."""
                rs_out = dram.tile([2, P, KS, HB], BF, tag=f"rso_{tagn}",
                                   name=f"rso_{tagn}")
                nc.gpsimd.collective_compute(
                    "ReduceScatter", OP.add, replica_groups=RG,
                    ins=[ar_in.opt()], outs=[rs_out.opt()],
                )
                rsret = own.tile([P, KS, TCC], BF, tag="rsret")
                for j in range(2):
                    nc.sync.dma_start(rsret[:, :, ds(j * HB, HB)], rs_out[j])
                nc.vector.tensor_tensor(x_own[:], x_own[:], rsret[:], OP.add)

            def rmsnorm_inplace(xc):
                """xc [P, KS, TOK] -> normed in place (ln folded into W)."""
                ssq = ps_c.tile([1, TOK], F32, tag="psc")
                for ks in range(KS):
                    xsq = scr.tile([P, TOK], BF, tag="xsq", bufs=3)
                    nc.vector.tensor_tensor(xsq[:], xc[:, ks, :], xc[:, ks, :],
                                            OP.mult)
                    nc.tensor.matmul(ssq[:], lhsT=ones_h[:], rhs=xsq[:],
                                     start=(ks == 0), stop=(ks == KS - 1))
                var = scr.tile([1, TOK], F32, tag="var", bufs=1)
                nc.vector.tensor_scalar(var[:], ssq[:], 1.0 / H, EPS,
                                        OP.mult, OP.add)
                rec = scr.tile([1, TOK], F32, tag="rec", bufs=1)
                nc.vector.reciprocal(rec[:], var[:])
                rstd = scr.tile([1, TOK], F32, tag="rstd", bufs=1)
                nc.scalar.activation(rstd[:], rec[:], AF.Sqrt)
                rb = ps_b.tile([P, TOK], F32, tag="psb")
                nc.tensor.matmul(rb[:], lhsT=ones_r[:], rhs=rstd[:],
                                 start=True, stop=True)
                for ks in range(KS):
                    nc.vector.tensor_tensor(xc[:, ks, :], xc[:, ks, :],
                                            rb[:], OP.mult)

            def load_chunk(agx, j, g, dst):
                """dst [P, KS, TOK] <- gathered half-j blocks of ranks 4g..4g+3."""
                for m in range(NB):
                    nc.sync.dma_start(dst[:, :, ds(m * HB, HB)],
                                      agx[j][NB * g + m])

            def rope_chunk(src_ps, j, g, dst):
                """dst = src*cos + rotate_half(src)*sin for chunk (j, g).
                src_ps [P, TOK] PSUM; dst [P, NB, HB] strided."""
                coss = cos_sb[:, ds(NB * g, NB), j, :]
                sins = sin_sb[:, ds(NB * g, NB), j, :]
                qtmp = scr.tile([P, NB, HB], BF, tag="qtmp")
                nc.vector.tensor_copy(qtmp[:], src_ps)
                rot = ps_b.tile([P, NB, HB], F32, tag="psb")
                nc.tensor.matmul(rot[:], lhsT=rt_sb[:], rhs=qtmp[:],
                                 start=True, stop=True)
                tsin = scr.tile([P, NB, HB], BF, tag="tsin")
                nc.vector.tensor_tensor(tsin[:], rot[:], sins, OP.mult)
                nc.vector.tensor_tensor(dst, qtmp[:], coss, OP.mult)
                nc.vector.tensor_tensor(dst, dst, tsin[:], OP.add)

            # token order: [rank, half, 128] == global
            qT = qkv.tile([P, QH, NCORES, 2, HB], BF)
            kT = qkv.tile([P, NCORES, 2, HB], BF)
            vaug = qkv.tile([P, NKC, HD + 1], BF)
            oT = qkv.tile([P, QH, S], BF)

            for l in range(L):
                wq_sb = wbig.tile([P, KS, QH * HD], BF, tag="wq")
                nc.sync.dma_start(wq_sb[:], wq_ap[l])
                wk_sb = wbig.tile([P, KS, HD], BF, tag="wk")
                nc.sync.dma_start(wk_sb[:], wk_ap[l])
                wv_sb = wbig.tile([P, KS, HD], BF, tag="wv")
                nc.sync.dma_start(wv_sb[:], wv_ap[l])
                wo_sb = wbig.tile([P, ODC, H], BF, tag="wo")
                nc.sync.dma_start(wo_sb[:], wo_ap[l])

                nc.vector.memset(vaug[:, :, 0:1], 1.0)

                agx1 = gather_x(f"h1_{l}")

                # ---- qkv projections + rope, per (half, rank-group) chunk ----
                for j in range(2):
                    for g in range(2):
                        h1 = hpool.tile([P, KS, TOK], BF, tag="hch")
                        load_chunk(agx1, j, g, h1)
                        rmsnorm_inplace(h1)
                        for hh in range(QH):
                            qp = ps_a.tile([P, NB, HB], F32, tag="psa")
                            for ks in range(KS):
                                nc.tensor.matmul(
                                    qp[:], lhsT=wq_sb[:, ks, ds(hh * HD, HD)],
                                    rhs=h1[:, ks, :],
                                    start=(ks == 0), stop=(ks == KS - 1))
                            rope_chunk(qp[:], j, g,
                                       qT[:, hh, ds(NB * g, NB), j, :])
                        kp = ps_a.tile([P, NB, HB], F32, tag="psa")
                        for ks in range(KS):
                            nc.tensor.matmul(kp[:], lhsT=wk_sb[:, ks, :],
                                             rhs=h1[:, ks, :],
                                             start=(ks == 0), stop=(ks == KS - 1))
                        rope_chunk(kp[:], j, g, kT[:, ds(NB * g, NB), j, :])
                        for m in range(NB):
                            vp = ps_b.tile([P, TOK], F32, tag="psb")
                            for ks in range(KS):
                                nc.tensor.matmul(
                                    vp[:, :HD], lhsT=h1[:, ks, ds(m * HB, HB)],
                                    rhs=wv_sb[:, ks, :],
                                    start=(ks == 0), stop=(ks == KS - 1))
                            nc.vector.tensor_copy(
                                vaug[:, (NB * g + m) * 2 + j, 1:], vp[:, :HD])

                # ---- attention (2 local heads, full 2048x2048, no mask) ----
                for hh in range(QH):
                    for qa in range(NQA):
                        att = attp.tile([P, NKC, TOKA], BF, tag="att")
                        for kc in range(NKC):
                            sp = ps_a.tile([P, TOK], F32, tag="psa")
                            nc.tensor.matmul(sp[:, :TOKA],
                                             lhsT=kT[:, kc // 2, kc % 2, :],
                                             rhs=qT[:, hh, qa, :, :],
                                             start=True, stop=True)
                            nc.scalar.activation(att[:, kc, :], sp[:, :TOKA],
                                                 AF.Exp, scale=SCALE)
                        for m in range(TOKA // P):
                            op_ = ps_b.tile([P, TOK], F32, tag="psb")
                            for kc in range(NKC):
                                nc.tensor.matmul(op_[:, :HD + 1],
                                                 lhsT=att[:, kc, ds(m * P, P)],
                                                 rhs=vaug[:, kc, :],
                                                 start=(kc == 0),
                                                 stop=(kc == NKC - 1))
                            rcp = scr.tile([P, 1], F32, tag="rcp")
                            nc.vector.reciprocal(rcp[:], op_[:, 0:1])
                            onrm = scr.tile([P, P], BF, tag="onrm")
                            nc.vector.tensor_scalar_mul(onrm[:], op_[:, 1:HD + 1],
                                                        rcp[:, :1])
                            tp = ps_c.tile([P, P], BF, tag="psc")
                            nc.tensor.transpose(tp[:], onrm[:], ident[:])
                            nc.vector.tensor_copy(
                                oT[:, hh, ds(qa * TOKA + m * P, P)], tp[:])

                # ---- o-proj (row-parallel) -> ReduceScatter -> residual ----
                aro_in = dram.tile([NCORES, 2, P, KS, HB], BF, tag=f"aroi{l}",
                                   name=f"aroi{l}")
                for qc in range(NQC):
                    t0 = qc * TOK
                    arst = retp.tile([P, KS, NB, HB], BF, tag="ret")
                    for hc in range(KS):
                        xo = ps_a.tile([P, NB, HB], F32, tag="psa")
                        for odc in range(ODC):
                            nc.tensor.matmul(xo[:], lhsT=wo_sb[:, odc, ds(hc * P, P)],
                                             rhs=oT[:, odc, ds(t0, TOK)],
                                             start=(odc == 0), stop=(odc == ODC - 1))
                        nc.vector.tensor_copy(arst[:, hc, :, :], xo[:])
                    # chunk qc holds ranks 2qc, 2qc+1, both halves
                    for b in range(2):
                        for j in range(2):
                            nc.sync.dma_start(aro_in[2 * qc + b, j],
                                              arst[:, :, 2 * b + j, :])
                rs_and_add(aro_in, f"o{l}")

                agx2 = gather_x(f"h2_{l}")

                # ---- MLP (column-parallel gate/up, row-parallel down) ----
                ard_in = dram.tile([NCORES, 2, P, KS, HB], BF, tag=f"ardi{l}",
                                   name=f"ardi{l}")
                for j in range(2):
                    for g in range(2):
                        h2 = hpool.tile([P, KS, TOK], BF, tag="hch")
                        load_chunk(agx2, j, g, h2)
                        rmsnorm_inplace(h2)
                        act = actp.tile([P, IB, TOK], BF, tag="act")
                        for ib in range(IB):
                            gp = ps_a.tile([P, TOK], F32, tag="psa")
                            for kh in range(2):
                                wg_sb = wstr.tile([P, KS // 2, HD], BF, tag="wg")
                                nc.sync.dma_start(
                                    wg_sb[:],
                                    wg_ap[l][:, ib, ds(kh * KS // 2, KS // 2)])
                                for k8 in range(KS // 2):
                                    ks = kh * (KS // 2) + k8
                                    nc.tensor.matmul(gp[:], lhsT=wg_sb[:, k8, :],
                                                     rhs=h2[:, ks, :],
                                                     start=(ks == 0),
                                                     stop=(ks == KS - 1))
                            up = ps_b.tile([P, TOK], F32, tag="psb")
                            for kh in range(2):
                                wu_sb = wstr.tile([P, KS // 2, HD], BF, tag="wu")
                                nc.sync.dma_start(
                                    wu_sb[:],
                                    wu_ap[l][:, ib, ds(kh * KS // 2, KS // 2)])
                                for k8 in range(KS // 2):
                                    ks = kh * (KS // 2) + k8
                                    nc.tensor.matmul(up[:], lhsT=wu_sb[:, k8, :],
                                                     rhs=h2[:, ks, :],
                                                     start=(ks == 0),
                                                     stop=(ks == KS - 1))
                            gs = scr.tile([P, TOK], BF, tag="xsq", bufs=3)
                            nc.scalar.activation(gs[:], gp[:], AF.Silu)
                            nc.vector.tensor_tensor(act[:, ib, :], gs[:], up[:],
                                                    OP.mult)
                        arst2 = retp.tile([P, KS, NB, HB], BF, tag="ret")
                        for gg in range(NHG):
                            wd_sb = wstr.tile([P, IB, WDC], BF, tag="wd")
                            nc.sync.dma_start(wd_sb[:], wd_ap[l][:, gg])
                            for hs in range(WDC // P):
                                hc = gg * (WDC // P) + hs
                                dx = ps_a.tile([P, NB, HB], F32, tag="psa")
                                for ib in range(IB):
                                    nc.tensor.matmul(
                                        dx[:], lhsT=wd_sb[:, ib, ds(hs * P, P)],
                                        rhs=act[:, ib, :],
                                        start=(ib == 0), stop=(ib == IB - 1))
                                nc.vector.tensor_copy(arst2[:, hc, :, :], dx[:])
                        for m in range(NB):
                            nc.sync.dma_start(ard_in[NB * g + m, j],
                                              arst2[:, :, m, :])
                rs_and_add(ard_in, f"d{l}")

            # ---- final rmsnorm on own tokens ----
            hf = own.tile([P, KS, TCC], BF, tag="hf")
            ssq = ps_c.tile([1, TOK], F32, tag="psc")
            for ks in range(KS):
                xsq = scr.tile([P, TOK], BF, tag="xsq", bufs=3)
                nc.vector.tensor_tensor(xsq[:, :TCC], x_own[:, ks, :],
                                        x_own[:, ks, :], OP.mult)
                nc.tensor.matmul(ssq[:, :TCC], lhsT=ones_h[:], rhs=xsq[:, :TCC],
                                 start=(ks == 0), stop=(ks == KS - 1))
            var = scr.tile([1, TOK], F32, tag="var", bufs=1)
            nc.vector.tensor_scalar(var[:, :TCC], ssq[:, :TCC], 1.0 / H, EPS,
                                    OP.mult, OP.add)
            rec = scr.tile([1, TOK], F32, tag="rec", bufs=1)
            nc.vector.reciprocal(rec[:, :TCC], var[:, :TCC])
            rstd = scr.tile([1, TOK], F32, tag="rstd", bufs=1)
            nc.scalar.activation(rstd[:, :TCC], rec[:, :TCC], AF.Sqrt)
            rb = ps_b.tile([P, TOK], F32, tag="psb")
            nc.tensor.matmul(rb[:, :TCC], lhsT=ones_r[:], rhs=rstd[:, :TCC],
                             start=True, stop=True)
            hf2 = hf
            for ks in range(KS):
                nc.vector.tensor_tensor(hf2[:, ks, :], x_own[:, ks, :],
                                        rb[:, :TCC], OP.mult)
            for ks in range(KS):
                fin = scr.tile([P, TCC], F32, tag="fin", bufs=1)
                nc.vector.tensor_scalar_mul(fin[:], hf2[:, ks, :],
                                            nw_sb[:, ds(ks, 1)])
                nc.sync.dma_start(out_ap[:, ks], fin[:])

    nc.compile()
    return nc


def _prep_inputs(input_ids, embed, Wq, Wk, Wv, Wo, Wg, Wu, Wd, ln1, ln2, norm_w):
    bf16 = ml_dtypes.bfloat16
    f32 = np.float32
    ids = np.asarray(input_ids).reshape(S)
    embed = np.asarray(embed, f32)
    ln1 = np.asarray(ln1, f32)
    ln2 = np.asarray(ln2, f32)

    # embedding gather on host; residual is H-major on device
    x0T = np.ascontiguousarray(embed[ids].T).reshape(KS, P, S)   # [KS, P, S] f32

    Wq = np.asarray(Wq, f32) * ln1[:, :, None]
    Wk = np.asarray(Wk, f32) * ln1[:, :, None]
    Wv = np.asarray(Wv, f32) * ln1[:, :, None]
    Wo = np.asarray(Wo, f32)
    Wg = np.asarray(Wg, f32) * ln2[:, :, None]
    Wu = np.asarray(Wu, f32) * ln2[:, :, None]
    Wd = np.asarray(Wd, f32)

    inv = 1.0 / (THETA ** (np.arange(0, HD, 2, dtype=np.float64) / HD))  # [64]
    fr = np.arange(S, dtype=np.float64)[None, :] * inv[:, None]          # [64, S]
    cosT = np.ascontiguousarray(
        np.concatenate([np.cos(fr), np.cos(fr)], 0).astype(bf16))        # [128, S]
    sinT = np.ascontiguousarray(
        np.concatenate([np.sin(fr), np.sin(fr)], 0).astype(bf16))

    rotT = np.zeros((P, P), f32)
    for m in range(HD // 2):
        rotT[m + HD // 2, m] = -1.0
    for m in range(HD // 2, HD):
        rotT[m - HD // 2, m] = 1.0
    rotT = rotT.astype(bf16)

    nwT = np.ascontiguousarray(np.asarray(norm_w, f32).reshape(KS, P).T)  # [P, KS]

    def colshard(w, lo, hi):
        # [L, H, N] cols [lo:hi) -> [L, P, KS, hi-lo]
        return np.ascontiguousarray(
            w[:, :, lo:hi].reshape(L, KS, P, hi - lo).transpose(0, 2, 1, 3)
        ).astype(bf16)

    in_maps = []
    for c in range(NCORES):
        wq = colshard(Wq, c * QH * HD, (c + 1) * QH * HD)
        wk = colshard(Wk, c * HD, (c + 1) * HD)
        wv = colshard(Wv, c * HD, (c + 1) * HD)
        # wo rows for this core's heads: [L, 256, H] -> [L, P, ODC, H]
        wo = np.ascontiguousarray(
            Wo[:, c * QH * HD:(c + 1) * QH * HD, :]
            .reshape(L, ODC, P, H).transpose(0, 2, 1, 3)).astype(bf16)
        # wg/wu cols for this core: [L, H, IC] -> [L, P, IB, KS, HD]
        wg = np.ascontiguousarray(
            Wg[:, :, c * IC:(c + 1) * IC]
            .reshape(L, KS, P, IB, HD).transpose(0, 2, 3, 1, 4)).astype(bf16)
        wu = np.ascontiguousarray(
            Wu[:, :, c * IC:(c + 1) * IC]
            .reshape(L, KS, P, IB, HD).transpose(0, 2, 3, 1, 4)).astype(bf16)
        # wd rows for this core: [L, IC, H] -> [L, P, NHG, IB, WDC]
        wd = np.ascontiguousarray(
            Wd[:, c * IC:(c + 1) * IC, :]
            .reshape(L, IB, P, NHG, WDC).transpose(0, 2, 3, 1, 4)).astype(bf16)
        x0 = np.ascontiguousarray(
            x0T[:, :, c * TCC:(c + 1) * TCC].transpose(1, 0, 2).astype(bf16))
        in_maps.append(dict(
            x0=x0, wq=wq, wk=wk, wv=wv, wo=wo, wg=wg, wu=wu, wd=wd,
            cosT=cosT, sinT=sinT, rotT=rotT, nwT=nwT,
        ))
    return in_maps


def _assemble(results):
    outT = np.empty((P, KS, S), np.float32)
    for c in range(NCORES):
        outT[:, :, c * TCC:(c + 1) * TCC] = np.asarray(results[c]["out"], np.float32)
    # [P, KS, S] -> [H, S] -> [S, H]
    return outT.transpose(1, 0, 2).reshape(H, S).T.copy().reshape(B, S, H)


def kernel(**inputs):
    global LAST_RESULT, LAST_NC, LAST_IN_MAPS
    in_maps = _prep_inputs(**inputs)
    nc = _build()
    res = bass_utils.run_bass_kernel_spmd(nc, in_maps, core_ids=list(range(NCORES)))
    LAST_RESULT = res
    LAST_NC = nc
    LAST_IN_MAPS = in_maps
    return _assemble(res.results)


# revision 5
# speedup vs baseline: 1.0794x; 1.0794x over previous
"""Llama-style 2-layer transformer forward, tensor-parallel + sequence-parallel
on 8 NeuronCores (Megatron-SP style), with pipelined half-AllGathers.

TP: q/k/v/gate/up column-sharded, o/down row-sharded; core c owns q heads
{2c, 2c+1}, kv head c, I-slice [c*1024,(c+1)*1024).
SP: the residual x lives sequence-sharded (core c holds its 256 tokens) in
H-major layout [128 H-partitions, 16 H-chunks, 256 tokens] bf16.

Boundary flow: the RAW residual shard is AllGathered in two 128-token halves
(so downstream compute starts as soon as the first half lands), each core
redundantly rmsnorms the gathered chunks in place, runs the column-parallel
matmuls, and the row-parallel o/down partial sums are ReduceScattered
(8MB -> 1MB) back onto the local shard.  RS+AG moves the same bytes as an
AllReduce but the RS output is 8x smaller and nothing sits serially behind
the gather except the RS itself.

All matmuls are transpose-free via the H-major layout: ones-matmul rmsnorm
statistics, a PE rotation matrix for rope, transposed scores with a
ones-column in V for the softmax denominator, weight-stationary row-parallel
projections.  Gathered-token chunks are (half j, rank-group g) sets of
4 x 128 strided tokens; tensors indexed [rank, half, 128] remain in global
token order because rank-major x half x token equals the global ordering.
"""

import numpy as np
import ml_dtypes

import concourse.bass as bass
import concourse.tile as tile
from concourse import bacc, mybir
from concourse import bass_utils
from concourse.bass import ds
from concourse.masks import make_identity

P = 128
B, S, H, NH, NKV, L, I, V = 1, 2048, 2048, 16, 8, 2, 8192, 32000
HD = H // NH            # 128
NCORES = 8
QH = NH // NCORES       # 2 q heads per core
IC = I // NCORES        # 1024 intermediate cols per core
ODC = QH                # o-proj contraction chunks of 128
IB = IC // P            # 8 I-blocks per core
KS = H // P             # 16 contraction subtiles over H
TCC = S // NCORES       # 256 own tokens
HB = TCC // 2           # 128-token AllGather half-blocks
TOK = 512               # token chunk for projections / MLP
NB = TOK // HB          # 4 rank-subblocks per compute chunk
NQC = S // TOK          # 4
TOKA = 256              # token chunk for attention scores
NQA = S // TOKA         # 8
NKC = S // P            # 16 key chunks
WDC = 256               # wd H-col streaming chunk
NHG = H // WDC          # 8 H-col groups for wd streaming
EPS = 1e-5
THETA = 10000.0
SCALE = HD ** -0.5

BF = mybir.dt.bfloat16
F32 = mybir.dt.float32
AF = mybir.ActivationFunctionType
OP = mybir.AluOpType

LAST_RESULT = None
LAST_NC = None
LAST_IN_MAPS = None


def _build():
    nc = bacc.Bacc("TRN2", target_bir_lowering=False, debug=False,
                   enable_asserts=False, num_devices=NCORES)

    x0_ap = nc.dram_tensor("x0", [P, KS, TCC], BF, kind="ExternalInput").ap()
    wq_ap = nc.dram_tensor("wq", [L, P, KS, QH * HD], BF, kind="ExternalInput").ap()
    wk_ap = nc.dram_tensor("wk", [L, P, KS, HD], BF, kind="ExternalInput").ap()
    wv_ap = nc.dram_tensor("wv", [L, P, KS, HD], BF, kind="ExternalInput").ap()
    wo_ap = nc.dram_tensor("wo", [L, P, ODC, H], BF, kind="ExternalInput").ap()
    wg_ap = nc.dram_tensor("wg", [L, P, IB, KS, HD], BF, kind="ExternalInput").ap()
    wu_ap = nc.dram_tensor("wu", [L, P, IB, KS, HD], BF, kind="ExternalInput").ap()
    wd_ap = nc.dram_tensor("wd", [L, P, NHG, IB, WDC], BF, kind="ExternalInput").ap()
    cos_ap = nc.dram_tensor("cosT", [P, S], BF, kind="ExternalInput").ap()
    sin_ap = nc.dram_tensor("sinT", [P, S], BF, kind="ExternalInput").ap()
    rt_ap = nc.dram_tensor("rotT", [P, P], BF, kind="ExternalInput").ap()
    nw_ap = nc.dram_tensor("nwT", [P, KS], F32, kind="ExternalInput").ap()
    out_ap = nc.dram_tensor("out", [P, KS, TCC], BF, kind="ExternalOutput").ap()

    RG = [list(range(NCORES))]

    with tile.TileContext(nc) as tc:
        with (
            tc.tile_pool(name="const", bufs=1) as const,
            tc.tile_pool(name="own", bufs=1) as own,
            tc.tile_pool(name="hch", bufs=2) as hpool,
            tc.tile_pool(name="retp", bufs=2) as retp,
            tc.tile_pool(name="qkv", bufs=1) as qkv,
            tc.tile_pool(name="attp", bufs=2) as attp,
            tc.tile_pool(name="actp", bufs=2) as actp,
            tc.tile_pool(name="wbig", bufs=1) as wbig,
            tc.tile_pool(name="wstr", bufs=2) as wstr,
            tc.tile_pool(name="scr", bufs=2) as scr,
            tc.tile_pool(name="ps_a", bufs=3, space="PSUM") as ps_a,
            tc.tile_pool(name="ps_b", bufs=3, space="PSUM") as ps_b,
            tc.tile_pool(name="ps_c", bufs=2, space="PSUM") as ps_c,
            tc.tile_pool(name="dram", bufs=1, space="DRAM") as dram,
        ):
            ident = const.tile([P, P], BF)
            make_identity(nc, ident[:])
            # cos/sin pre-arranged on host in (half j, group g) chunk order
            cos_sb = const.tile([P, 2, 2, NB, HB], BF)
            nc.sync.dma_start(cos_sb[:], cos_ap[:])
            sin_sb = const.tile([P, 2, 2, NB, HB], BF)
            nc.sync.dma_start(sin_sb[:], sin_ap[:])
            rt_sb = const.tile([P, P], BF)
            nc.sync.dma_start(rt_sb[:], rt_ap[:])
            nw_sb = const.tile([P, KS], F32)
            nc.sync.dma_start(nw_sb[:], nw_ap[:])
            ones_h = const.tile([P, 1], BF)
            nc.vector.memset(ones_h[:], 1.0)
            ones_r = const.tile([1, P], F32)
            nc.vector.memset(ones_r[:], 1.0)

            x_own = own.tile([P, KS, TCC], BF)
            nc.sync.dma_start(x_own[:], x0_ap[:])

            def gather_x(tagn):
                """AllGather x_own in two 128-token halves -> per-half
                [NCORES, P, KS, HB] shared DRAM tiles."""
                agx_in = dram.tile([2, P, KS, HB], BF, tag=f"agi_{tagn}",
                                   name=f"agi_{tagn}")
                outs = []
                for j in range(2):
                    nc.sync.dma_start(agx_in[j], x_own[:, :, ds(j * HB, HB)])
                    ago = dram.tile([NCORES, P, KS, HB], BF,
                                    tag=f"ago_{tagn}_{j}", name=f"ago_{tagn}_{j}",
                                    addr_space="Shared")
                    nc.gpsimd.collective_compute(
                        "AllGather", OP.bypass, replica_groups=RG,
                        ins=[agx_in[j].opt()], outs=[ago.opt()],
                    )
                    outs.append(ago)
                return outs

            def rs_and_add(ar_in, tagn):
                """ReduceScatter staged partials, add own block to x_own.
                ar_in layout [NCORES, 2, P, KS, HB] (rank-major, half-minor)."""
                rs_out = dram.tile([2, P, KS, HB], BF, tag=f"rso_{tagn}",
                                   name=f"rso_{tagn}")
                nc.gpsimd.collective_compute(
                    "ReduceScatter", OP.add, replica_groups=RG,
                    ins=[ar_in.opt()], outs=[rs_out.opt()],
                )
                rsret = own.tile([P, KS, TCC], BF, tag="rsret")
                for j in range(2):
                    nc.sync.dma_start(rsret[:, :, ds(j * HB, HB)], rs_out[j])
                nc.vector.tensor_tensor(x_own[:], x_own[:], rsret[:], OP.add)

            def rmsnorm_inplace(xc):
                """xc [P, KS, TOK] -> normed in place (ln folded into W)."""
                ssq = ps_c.tile([1, TOK], F32, tag="psc")
                for ks in range(KS):
                    xsq = scr.tile([P, TOK], BF, tag="xsq", bufs=3)
                    nc.vector.tensor_tensor(xsq[:], xc[:, ks, :], xc[:, ks, :],
                                            OP.mult)
                    nc.tensor.matmul(ssq[:], lhsT=ones_h[:], rhs=xsq[:],
                                     start=(ks == 0), stop=(ks == KS - 1))
                var = scr.tile([1, TOK], F32, tag="var", bufs=1)
                nc.vector.tensor_scalar(var[:], ssq[:], 1.0 / H, EPS,
                                        OP.mult, OP.add)
                rec = scr.tile([1, TOK], F32, tag="rec", bufs=1)
                nc.vector.reciprocal(rec[:], var[:])
                rstd = scr.tile([1, TOK], F32, tag="rstd", bufs=1)
                nc.scalar.activation(rstd[:], rec[:], AF.Sqrt)
                rb = ps_b.tile([P, TOK], F32, tag="psb")
                nc.tensor.matmul(rb[:], lhsT=ones_r[:], rhs=rstd[:],
                                 start=True, stop=True)
                for ks in range(KS):
                    nc.vector.tensor_tensor(xc[:, ks, :], xc[:, ks, :],
                                            rb[:], OP.mult)

            def load_chunk(agx, j, g, dst):
                """dst [P, KS, TOK] <- gathered half-j blocks of ranks 4g..4g+3."""
                for m in range(NB):
                    nc.sync.dma_start(dst[:, :, ds(m * HB, HB)],
                                      agx[j][NB * g + m])

            def rope_chunk(src_ps, j, g, dst):
                """dst = src*cos + rotate_half(src)*sin for chunk (j, g).
                src_ps [P, TOK] PSUM; dst [P, NB, HB] strided."""
                coss = cos_sb[:, j, g, :, :]
                sins = sin_sb[:, j, g, :, :]
                qtmp = scr.tile([P, NB, HB], BF, tag="qtmp")
                nc.vector.tensor_copy(qtmp[:], src_ps)
                rot = ps_b.tile([P, NB, HB], F32, tag="psb")
                nc.tensor.matmul(rot[:], lhsT=rt_sb[:], rhs=qtmp[:],
                                 start=True, stop=True)
                tsin = scr.tile([P, NB, HB], BF, tag="tsin")
                nc.vector.tensor_tensor(tsin[:], rot[:], sins, OP.mult)
                nc.vector.tensor_tensor(dst, qtmp[:], coss, OP.mult)
                nc.vector.tensor_tensor(dst, dst, tsin[:], OP.add)

            # token order: [rank, half, 128] == global
            qT = qkv.tile([P, QH, NCORES, 2, HB], BF)
            kT = qkv.tile([P, NCORES, 2, HB], BF)
            vaug = qkv.tile([P, NKC, HD + 1], BF)
            oT = qkv.tile([P, QH, S], BF)

            for l in range(L):
                wq_sb = wbig.tile([P, KS, QH * HD], BF, tag="wq")
                nc.sync.dma_start(wq_sb[:], wq_ap[l])
                wk_sb = wbig.tile([P, KS, HD], BF, tag="wk")
                nc.sync.dma_start(wk_sb[:], wk_ap[l])
                wv_sb = wbig.tile([P, KS, HD], BF, tag="wv")
                nc.sync.dma_start(wv_sb[:], wv_ap[l])
                wo_sb = wbig.tile([P, ODC, H], BF, tag="wo")
                nc.sync.dma_start(wo_sb[:], wo_ap[l])

                nc.vector.memset(vaug[:, :, 0:1], 1.0)

                agx1 = gather_x(f"h1_{l}")

                # ---- qkv projections + rope, per (half, rank-group) chunk ----
                for j in range(2):
                    for g in range(2):
                        h1 = hpool.tile([P, KS, TOK], BF, tag="hch")
                        load_chunk(agx1, j, g, h1)
                        rmsnorm_inplace(h1)
                        for hh in range(QH):
                            qp = ps_a.tile([P, NB, HB], F32, tag="psa")
                            for ks in range(KS):
                                nc.tensor.matmul(
                                    qp[:], lhsT=wq_sb[:, ks, ds(hh * HD, HD)],
                                    rhs=h1[:, ks, :],
                                    start=(ks == 0), stop=(ks == KS - 1))
                            rope_chunk(qp[:], j, g,
                                       qT[:, hh, ds(NB * g, NB), j, :])
                        kp = ps_a.tile([P, NB, HB], F32, tag="psa")
                        for ks in range(KS):
                            nc.tensor.matmul(kp[:], lhsT=wk_sb[:, ks, :],
                                             rhs=h1[:, ks, :],
                                             start=(ks == 0), stop=(ks == KS - 1))
                        rope_chunk(kp[:], j, g, kT[:, ds(NB * g, NB), j, :])
                        for m in range(NB):
                            vp = ps_b.tile([P, TOK], F32, tag="psb")
                            for ks in range(KS):
                                nc.tensor.matmul(
                                    vp[:, :HD], lhsT=h1[:, ks, ds(m * HB, HB)],
                                    rhs=wv_sb[:, ks, :],
                                    start=(ks == 0), stop=(ks == KS - 1))
                            nc.vector.tensor_copy(
                                vaug[:, (NB * g + m) * 2 + j, 1:], vp[:, :HD])

                # ---- attention (2 local heads, full 2048x2048, no mask) ----
                for hh in range(QH):
                    for qa in range(NQA):
                        att = attp.tile([P, NKC, TOKA], BF, tag="att")
                        for kc in range(NKC):
                            sp = ps_a.tile([P, TOK], F32, tag="psa")
                            nc.tensor.matmul(sp[:, :TOKA],
                                             lhsT=kT[:, kc // 2, kc % 2, :],
                                             rhs=qT[:, hh, qa, :, :],
                                             start=True, stop=True)
                            nc.scalar.activation(att[:, kc, :], sp[:, :TOKA],
                                                 AF.Exp, scale=SCALE)
                        for m in range(TOKA // P):
                            op_ = ps_b.tile([P, TOK], F32, tag="psb")
                            for kc in range(NKC):
                                nc.tensor.matmul(op_[:, :HD + 1],
                                                 lhsT=att[:, kc, ds(m * P, P)],
                                                 rhs=vaug[:, kc, :],
                                                 start=(kc == 0),
                                                 stop=(kc == NKC - 1))
                            rcp = scr.tile([P, 1], F32, tag="rcp")
                            nc.vector.reciprocal(rcp[:], op_[:, 0:1])
                            onrm = scr.tile([P, P], BF, tag="onrm")
                            nc.vector.tensor_scalar_mul(onrm[:], op_[:, 1:HD + 1],
                                                        rcp[:, :1])
                            tp = ps_c.tile([P, P], BF, tag="psc")
                            nc.tensor.transpose(tp[:], onrm[:], ident[:])
                            nc.vector.tensor_copy(
                                oT[:, hh, ds(qa * TOKA + m * P, P)], tp[:])

                # ---- o-proj (row-parallel) -> ReduceScatter -> residual ----
                aro_in = dram.tile([NCORES, 2, P, KS, HB], BF, tag=f"aroi{l}",
                                   name=f"aroi{l}")
                for qc in range(NQC):
                    t0 = qc * TOK
                    arst = retp.tile([P, KS, NB, HB], BF, tag="ret")
                    for hc in range(KS):
                        xo = ps_a.tile([P, NB, HB], F32, tag="psa")
                        for odc in range(ODC):
                            nc.tensor.matmul(xo[:], lhsT=wo_sb[:, odc, ds(hc * P, P)],
                                             rhs=oT[:, odc, ds(t0, TOK)],
                                             start=(odc == 0), stop=(odc == ODC - 1))
                        nc.vector.tensor_copy(arst[:, hc, :, :], xo[:])
                    # chunk qc holds ranks 2qc, 2qc+1, both halves
                    for b in range(2):
                        for j in range(2):
                            nc.sync.dma_start(aro_in[2 * qc + b, j],
                                              arst[:, :, 2 * b + j, :])
                rs_and_add(aro_in, f"o{l}")

                agx2 = gather_x(f"h2_{l}")

                # ---- MLP (column-parallel gate/up, row-parallel down) ----
                ard_in = dram.tile([NCORES, 2, P, KS, HB], BF, tag=f"ardi{l}",
                                   name=f"ardi{l}")
                for j in range(2):
                    for g in range(2):
                        h2 = hpool.tile([P, KS, TOK], BF, tag="hch")
                        load_chunk(agx2, j, g, h2)
                        rmsnorm_inplace(h2)
                        act = actp.tile([P, IB, TOK], BF, tag="act")
                        for ib in range(IB):
                            gp = ps_a.tile([P, TOK], F32, tag="psa")
                            for kh in range(2):
                                wg_sb = wstr.tile([P, KS // 2, HD], BF, tag="wg")
                                nc.sync.dma_start(
                                    wg_sb[:],
                                    wg_ap[l][:, ib, ds(kh * KS // 2, KS // 2)])
                                for k8 in range(KS // 2):
                                    ks = kh * (KS // 2) + k8
                                    nc.tensor.matmul(gp[:], lhsT=wg_sb[:, k8, :],
                                                     rhs=h2[:, ks, :],
                                                     start=(ks == 0),
                                                     stop=(ks == KS - 1))
                            up = ps_b.tile([P, TOK], F32, tag="psb")
                            for kh in range(2):
                                wu_sb = wstr.tile([P, KS // 2, HD], BF, tag="wu")
                                nc.sync.dma_start(
                                    wu_sb[:],
                                    wu_ap[l][:, ib, ds(kh * KS // 2, KS // 2)])
                                for k8 in range(KS // 2):
                                    ks = kh * (KS // 2) + k8
                                    nc.tensor.matmul(up[:], lhsT=wu_sb[:, k8, :],
                                                     rhs=h2[:, ks, :],
                                                     start=(ks == 0),
                                                     stop=(ks == KS - 1))
                            gs = scr.tile([P, TOK], BF, tag="xsq", bufs=3)
                            nc.scalar.activation(gs[:], gp[:], AF.Silu)
                            nc.vector.tensor_tensor(act[:, ib, :], gs[:], up[:],
                                                    OP.mult)
                        arst2 = retp.tile([P, KS, NB, HB], BF, tag="ret")
                        for gg in range(NHG):
                            wd_sb = wstr.tile([P, IB, WDC], BF, tag="wd")
                            nc.sync.dma_start(wd_sb[:], wd_ap[l][:, gg])
                            for hs in range(WDC // P):
                                hc = gg * (WDC // P) + hs
                                dx = ps_a.tile([P, NB, HB], F32, tag="psa")
                                for ib in range(IB):
                                    nc.tensor.matmul(
                                        dx[:], lhsT=wd_sb[:, ib, ds(hs * P, P)],
                                        rhs=act[:, ib, :],
                                        start=(ib == 0), stop=(ib == IB - 1))
                                nc.vector.tensor_copy(arst2[:, hc, :, :], dx[:])
                        for m in range(NB):
                            nc.sync.dma_start(ard_in[NB * g + m, j],
                                              arst2[:, :, m, :])
                rs_and_add(ard_in, f"d{l}")

            # ---- final rmsnorm on own tokens ----
            hf = own.tile([P, KS, TCC], BF, tag="hf")
            ssq = ps_c.tile([1, TOK], F32, tag="psc")
            for ks in range(KS):
                xsq = scr.tile([P, TOK], BF, tag="xsq", bufs=3)
                nc.vector.tensor_tensor(xsq[:, :TCC], x_own[:, ks, :],
                                        x_own[:, ks, :], OP.mult)
                nc.tensor.matmul(ssq[:, :TCC], lhsT=ones_h[:], rhs=xsq[:, :TCC],
                                 start=(ks == 0), stop=(ks == KS - 1))
            var = scr.tile([1, TOK], F32, tag="var", bufs=1)
            nc.vector.tensor_scalar(var[:, :TCC], ssq[:, :TCC], 1.0 / H, EPS,
                                    OP.mult, OP.add)
            rec = scr.tile([1, TOK], F32, tag="rec", bufs=1)
            nc.vector.reciprocal(rec[:, :TCC], var[:, :TCC])
            rstd = scr.tile([1, TOK], F32, tag="rstd", bufs=1)
            nc.scalar.activation(rstd[:, :TCC], rec[:, :TCC], AF.Sqrt)
            rb = ps_b.tile([P, TOK], F32, tag="psb")
            nc.tensor.matmul(rb[:, :TCC], lhsT=ones_r[:], rhs=rstd[:, :TCC],
                             start=True, stop=True)
            hf2 = hf
            for ks in range(KS):
                nc.vector.tensor_tensor(hf2[:, ks, :], x_own[:, ks, :],
                                        rb[:, :TCC], OP.mult)
            for ks in range(KS):
                fin = scr.tile([P, TCC], BF, tag="fin", bufs=1)
                nc.vector.tensor_scalar_mul(fin[:], hf2[:, ks, :],
                                            nw_sb[:, ds(ks, 1)])
                nc.sync.dma_start(out_ap[:, ks], fin[:])

    nc.compile()
    return nc


def _prep_inputs(input_ids, embed, Wq, Wk, Wv, Wo, Wg, Wu, Wd, ln1, ln2, norm_w):
    bf16 = ml_dtypes.bfloat16
    f32 = np.float32
    ids = np.asarray(input_ids).reshape(S)
    embed = np.asarray(embed, f32)
    ln1 = np.asarray(ln1, f32)
    ln2 = np.asarray(ln2, f32)

    # embedding gather on host; residual is H-major on device
    x0T = np.ascontiguousarray(embed[ids].T).reshape(KS, P, S)   # [KS, P, S] f32

    Wq = np.asarray(Wq, f32) * ln1[:, :, None]
    Wk = np.asarray(Wk, f32) * ln1[:, :, None]
    Wv = np.asarray(Wv, f32) * ln1[:, :, None]
    Wo = np.asarray(Wo, f32)
    Wg = np.asarray(Wg, f32) * ln2[:, :, None]
    Wu = np.asarray(Wu, f32) * ln2[:, :, None]
    Wd = np.asarray(Wd, f32)

    inv = 1.0 / (THETA ** (np.arange(0, HD, 2, dtype=np.float64) / HD))  # [64]
    fr = np.arange(S, dtype=np.float64)[None, :] * inv[:, None]          # [64, S]
    def chunk_order(t):
        # [P, S] -> [P, half j, group g, subblock m, 128] matching rope chunks
        return np.ascontiguousarray(
            t.reshape(P, 2, NB, 2, HB).transpose(0, 3, 1, 2, 4))
    cosT = chunk_order(np.concatenate([np.cos(fr), np.cos(fr)], 0)
                       ).astype(bf16).reshape(P, S)
    sinT = chunk_order(np.concatenate([np.sin(fr), np.sin(fr)], 0)
                       ).astype(bf16).reshape(P, S)

    rotT = np.zeros((P, P), f32)
    for m in range(HD // 2):
        rotT[m + HD // 2, m] = -1.0
    for m in range(HD // 2, HD):
        rotT[m - HD // 2, m] = 1.0
    rotT = rotT.astype(bf16)

    nwT = np.ascontiguousarray(np.asarray(norm_w, f32).reshape(KS, P).T)  # [P, KS]

    def colshard(w, lo, hi):
        # [L, H, N] cols [lo:hi) -> [L, P, KS, hi-lo]
        return np.ascontiguousarray(
            w[:, :, lo:hi].reshape(L, KS, P, hi - lo).transpose(0, 2, 1, 3)
        ).astype(bf16)

    in_maps = []
    for c in range(NCORES):
        wq = colshard(Wq, c * QH * HD, (c + 1) * QH * HD)
        wk = colshard(Wk, c * HD, (c + 1) * HD)
        wv = colshard(Wv, c * HD, (c + 1) * HD)
        # wo rows for this core's heads: [L, 256, H] -> [L, P, ODC, H]
        wo = np.ascontiguousarray(
            Wo[:, c * QH * HD:(c + 1) * QH * HD, :]
            .reshape(L, ODC, P, H).transpose(0, 2, 1, 3)).astype(bf16)
        # wg/wu cols for this core: [L, H, IC] -> [L, P, IB, KS, HD]
        wg = np.ascontiguousarray(
            Wg[:, :, c * IC:(c + 1) * IC]
            .reshape(L, KS, P, IB, HD).transpose(0, 2, 3, 1, 4)).astype(bf16)
        wu = np.ascontiguousarray(
            Wu[:, :, c * IC:(c + 1) * IC]
            .reshape(L, KS, P, IB, HD).transpose(0, 2, 3, 1, 4)).astype(bf16)
        # wd rows for this core: [L, IC, H] -> [L, P, NHG, IB, WDC]
        wd = np.ascontiguousarray(
            Wd[:, c * IC:(c + 1) * IC, :]
            .reshape(L, IB, P, NHG, WDC).transpose(0, 2, 3, 1, 4)).astype(bf16)
        x0 = np.ascontiguousarray(
            x0T[:, :, c * TCC:(c + 1) * TCC].transpose(1, 0, 2).astype(bf16))
        in_maps.append(dict(
            x0=x0, wq=wq, wk=wk, wv=wv, wo=wo, wg=wg, wu=wu, wd=wd,
            cosT=cosT, sinT=sinT, rotT=rotT, nwT=nwT,
        ))
    return in_maps


def _assemble(results):
    outT = np.empty((P, KS, S), np.float32)
    for c in range(NCORES):
        outT[:, :, c * TCC:(c + 1) * TCC] = np.asarray(
            results[c]["out"]).astype(np.float32)
    # [P, KS, S] -> [H, S] -> [S, H]
    return outT.transpose(1, 0, 2).reshape(H, S).T.copy().reshape(B, S, H)


def kernel(**inputs):
    global LAST_RESULT, LAST_NC, LAST_IN_MAPS
    in_maps = _prep_inputs(**inputs)
    nc = _build()
    res = bass_utils.run_bass_kernel_spmd(nc, in_maps, core_ids=list(range(NCORES)))
    LAST_RESULT = res
    LAST_NC = nc
    LAST_IN_MAPS = in_maps
    return _assemble(res.results)


# revision 6
# speedup vs baseline: 1.8452x; 1.7094x over previous
"""Llama-style 2-layer transformer forward, tensor-parallel + sequence-parallel
on 8 NeuronCores (Megatron-SP style), with pipelined half-AllGathers.

TP: q/k/v/gate/up column-sharded, o/down row-sharded; core c owns q heads
{2c, 2c+1}, kv head c, I-slice [c*1024,(c+1)*1024).
SP: the residual x lives sequence-sharded (core c holds its 256 tokens) in
H-major layout [128 H-partitions, 16 H-chunks, 256 tokens] bf16.

Boundary flow: the RAW residual shard is AllGathered in two 128-token halves
(so downstream compute starts as soon as the first half lands), each core
redundantly rmsnorms the gathered chunks in place, runs the column-parallel
matmuls, and the row-parallel o/down partial sums are ReduceScattered
(8MB -> 1MB) back onto the local shard.  RS+AG moves the same bytes as an
AllReduce but the RS output is 8x smaller and nothing sits serially behind
the gather except the RS itself.

All matmuls are transpose-free via the H-major layout: ones-matmul rmsnorm
statistics, a PE rotation matrix for rope, transposed scores with a
ones-column in V for the softmax denominator, weight-stationary row-parallel
projections.  Gathered-token chunks are (half j, rank-group g) sets of
4 x 128 strided tokens; tensors indexed [rank, half, 128] remain in global
token order because rank-major x half x token equals the global ordering.
"""

import numpy as np
import ml_dtypes

import concourse.bass as bass
import concourse.tile as tile
from concourse import bacc, mybir
from concourse import bass_utils
from concourse.bass import ds
from concourse.masks import make_identity

P = 128
B, S, H, NH, NKV, L, I, V = 1, 2048, 2048, 16, 8, 2, 8192, 32000
HD = H // NH            # 128
NCORES = 8
QH = NH // NCORES       # 2 q heads per core
IC = I // NCORES        # 1024 intermediate cols per core
ODC = QH                # o-proj contraction chunks of 128
IB = IC // P            # 8 I-blocks per core
KS = H // P             # 16 contraction subtiles over H
TCC = S // NCORES       # 256 own tokens
HB = TCC // 2           # 128-token AllGather half-blocks
TOK = 512               # token chunk for projections / MLP
NB = TOK // HB          # 4 rank-subblocks per compute chunk
NQC = S // TOK          # 4
TOKA = 256              # token chunk for attention scores
NQA = S // TOKA         # 8
NKC = S // P            # 16 key chunks
WDC = 256               # wd H-col streaming chunk
NHG = H // WDC          # 8 H-col groups for wd streaming
EPS = 1e-5
THETA = 10000.0
SCALE = HD ** -0.5

BF = mybir.dt.bfloat16
F32 = mybir.dt.float32
AF = mybir.ActivationFunctionType
OP = mybir.AluOpType

LAST_RESULT = None
LAST_NC = None
LAST_IN_MAPS = None


def _build():
    nc = bacc.Bacc("TRN2", target_bir_lowering=False, debug=False,
                   enable_asserts=False, num_devices=NCORES)

    x0_ap = nc.dram_tensor("x0", [P, KS, TCC], BF, kind="ExternalInput").ap()
    wq_ap = nc.dram_tensor("wq", [L, P, KS, QH * HD], BF, kind="ExternalInput").ap()
    wk_ap = nc.dram_tensor("wk", [L, P, KS, HD], BF, kind="ExternalInput").ap()
    wv_ap = nc.dram_tensor("wv", [L, P, KS, HD], BF, kind="ExternalInput").ap()
    wo_ap = nc.dram_tensor("wo", [L, P, ODC, H], BF, kind="ExternalInput").ap()
    wg_ap = nc.dram_tensor("wg", [L, P, IB, KS, HD], BF, kind="ExternalInput").ap()
    wu_ap = nc.dram_tensor("wu", [L, P, IB, KS, HD], BF, kind="ExternalInput").ap()
    wd_ap = nc.dram_tensor("wd", [L, P, NHG, IB, WDC], BF, kind="ExternalInput").ap()
    cos_ap = nc.dram_tensor("cosT", [P, S], BF, kind="ExternalInput").ap()
    sin_ap = nc.dram_tensor("sinT", [P, S], BF, kind="ExternalInput").ap()
    rt_ap = nc.dram_tensor("rotT", [P, P], BF, kind="ExternalInput").ap()
    nw_ap = nc.dram_tensor("nwT", [P, KS], F32, kind="ExternalInput").ap()
    out_ap = nc.dram_tensor("out", [P, KS, TCC], BF, kind="ExternalOutput").ap()

    RG = [list(range(NCORES))]

    with tile.TileContext(nc) as tc:
        with (
            tc.tile_pool(name="const", bufs=1) as const,
            tc.tile_pool(name="own", bufs=1) as own,
            tc.tile_pool(name="hch", bufs=2) as hpool,
            tc.tile_pool(name="retp", bufs=2) as retp,
            tc.tile_pool(name="qkv", bufs=1) as qkv,
            tc.tile_pool(name="attp", bufs=2) as attp,
            tc.tile_pool(name="actp", bufs=2) as actp,
            tc.tile_pool(name="wbig", bufs=1) as wbig,
            tc.tile_pool(name="wstr", bufs=2) as wstr,
            tc.tile_pool(name="scr", bufs=2) as scr,
            tc.tile_pool(name="ps_a", bufs=3, space="PSUM") as ps_a,
            tc.tile_pool(name="ps_b", bufs=3, space="PSUM") as ps_b,
            tc.tile_pool(name="ps_c", bufs=2, space="PSUM") as ps_c,
            tc.tile_pool(name="dram", bufs=1, space="DRAM") as dram,
        ):
            ident = const.tile([P, P], BF)
            make_identity(nc, ident[:])
            # cos/sin pre-arranged on host in (half j, group g) chunk order
            cos_sb = const.tile([P, 2, 2, NB, HB], BF)
            nc.sync.dma_start(cos_sb[:], cos_ap[:])
            sin_sb = const.tile([P, 2, 2, NB, HB], BF)
            nc.sync.dma_start(sin_sb[:], sin_ap[:])
            rt_sb = const.tile([P, P], BF)
            nc.sync.dma_start(rt_sb[:], rt_ap[:])
            nw_sb = const.tile([P, KS], F32)
            nc.sync.dma_start(nw_sb[:], nw_ap[:])
            ones_h = const.tile([P, 1], BF)
            nc.vector.memset(ones_h[:], 1.0)
            ones_r = const.tile([1, P], F32)
            nc.vector.memset(ones_r[:], 1.0)

            x_own = own.tile([P, KS, TCC], BF)
            nc.sync.dma_start(x_own[:], x0_ap[:])

            def gather_x(tagn):
                """AllGather x_own in two 128-token halves -> per-half
                [NCORES, P, KS, HB] shared DRAM tiles."""
                agx_in = dram.tile([2, P, KS, HB], BF, tag=f"agi_{tagn}",
                                   name=f"agi_{tagn}")
                outs = []
                for j in range(2):
                    nc.sync.dma_start(agx_in[j], x_own[:, :, ds(j * HB, HB)])
                    ago = dram.tile([NCORES, P, KS, HB], BF,
                                    tag=f"ago_{tagn}_{j}", name=f"ago_{tagn}_{j}",
                                    addr_space="Shared")
                    nc.gpsimd.collective_compute(
                        "AllGather", OP.bypass, replica_groups=RG,
                        ins=[agx_in[j].opt()], outs=[ago.opt()],
                    )
                    outs.append(ago)
                return outs

            def rs_and_add(ar_in, tagn):
                """ReduceScatter staged partials, add own block to x_own.
                ar_in layout [NCORES, 2, P, KS, HB] (rank-major, half-minor)."""
                rs_out = dram.tile([2, P, KS, HB], BF, tag=f"rso_{tagn}",
                                   name=f"rso_{tagn}")
                nc.gpsimd.collective_compute(
                    "ReduceScatter", OP.add, replica_groups=RG,
                    ins=[ar_in.opt()], outs=[rs_out.opt()],
                )
                rsret = own.tile([P, KS, TCC], BF, tag="rsret")
                for j in range(2):
                    nc.sync.dma_start(rsret[:, :, ds(j * HB, HB)], rs_out[j])
                nc.vector.tensor_tensor(x_own[:], x_own[:], rsret[:], OP.add)

            def rmsnorm_inplace(xc):
                """xc [P, KS, TOK] -> normed in place (ln folded into W)."""
                ssq = ps_c.tile([1, TOK], F32, tag="psc")
                for ks in range(KS):
                    xsq = scr.tile([P, TOK], BF, tag="xsq", bufs=3)
                    nc.vector.tensor_tensor(xsq[:], xc[:, ks, :], xc[:, ks, :],
                                            OP.mult)
                    nc.tensor.matmul(ssq[:], lhsT=ones_h[:], rhs=xsq[:],
                                     start=(ks == 0), stop=(ks == KS - 1))
                var = scr.tile([1, TOK], F32, tag="var", bufs=1)
                nc.vector.tensor_scalar(var[:], ssq[:], 1.0 / H, EPS,
                                        OP.mult, OP.add)
                rec = scr.tile([1, TOK], F32, tag="rec", bufs=1)
                nc.vector.reciprocal(rec[:], var[:])
                rstd = scr.tile([1, TOK], F32, tag="rstd", bufs=1)
                nc.scalar.activation(rstd[:], rec[:], AF.Sqrt)
                rb = ps_b.tile([P, TOK], F32, tag="psb")
                nc.tensor.matmul(rb[:], lhsT=ones_r[:], rhs=rstd[:],
                                 start=True, stop=True)
                for ks in range(KS):
                    nc.vector.tensor_tensor(xc[:, ks, :], xc[:, ks, :],
                                            rb[:], OP.mult)

            def load_chunk(agx, j, g, dst):
                """dst [P, KS, TOK] <- gathered half-j blocks of ranks 4g..4g+3."""
                for m in range(NB):
                    nc.sync.dma_start(dst[:, :, ds(m * HB, HB)],
                                      agx[j][NB * g + m])

            def rope_chunk(src_ps, j, g, dst):
                """dst = src*cos + rotate_half(src)*sin for chunk (j, g).
                src_ps [P, TOK] PSUM; dst [P, NB, HB] strided."""
                coss = cos_sb[:, j, g, :, :]
                sins = sin_sb[:, j, g, :, :]
                qtmp = scr.tile([P, NB, HB], BF, tag="qtmp")
                nc.vector.tensor_copy(qtmp[:], src_ps)
                rot = ps_b.tile([P, NB, HB], F32, tag="psb")
                nc.tensor.matmul(rot[:], lhsT=rt_sb[:], rhs=qtmp[:],
                                 start=True, stop=True)
                tsin = scr.tile([P, NB, HB], BF, tag="tsin")
                nc.vector.tensor_tensor(tsin[:], rot[:], sins, OP.mult)
                nc.vector.tensor_tensor(dst, qtmp[:], coss, OP.mult)
                nc.vector.tensor_tensor(dst, dst, tsin[:], OP.add)

            # token order: [rank, half, 128] == global
            qT = qkv.tile([P, QH, NCORES, 2, HB], BF)
            kT = qkv.tile([P, NCORES, 2, HB], BF)
            vaug = qkv.tile([P, NKC, HD + 1], BF)
            oT = qkv.tile([P, QH, S], BF)

            for l in range(L):
                wq_sb = wbig.tile([P, KS, QH * HD], BF, tag="wq")
                nc.sync.dma_start(wq_sb[:], wq_ap[l])
                wk_sb = wbig.tile([P, KS, HD], BF, tag="wk")
                nc.sync.dma_start(wk_sb[:], wk_ap[l])
                wv_sb = wbig.tile([P, KS, HD], BF, tag="wv")
                nc.sync.dma_start(wv_sb[:], wv_ap[l])
                wo_sb = wbig.tile([P, ODC, H], BF, tag="wo")
                nc.sync.dma_start(wo_sb[:], wo_ap[l])

                nc.vector.memset(vaug[:, :, 0:1], 1.0)

                agx1 = gather_x(f"h1_{l}")

                # ---- qkv projections + rope, per (half, rank-group) chunk ----
                for j in range(2):
                    for g in range(2):
                        h1 = hpool.tile([P, KS, TOK], BF, tag="hch")
                        load_chunk(agx1, j, g, h1)
                        rmsnorm_inplace(h1)
                        for hh in range(QH):
                            qp = ps_a.tile([P, NB, HB], F32, tag="psa")
                            for ks in range(KS):
                                nc.tensor.matmul(
                                    qp[:], lhsT=wq_sb[:, ks, ds(hh * HD, HD)],
                                    rhs=h1[:, ks, :],
                                    start=(ks == 0), stop=(ks == KS - 1))
                            rope_chunk(qp[:], j, g,
                                       qT[:, hh, ds(NB * g, NB), j, :])
                        kp = ps_a.tile([P, NB, HB], F32, tag="psa")
                        for ks in range(KS):
                            nc.tensor.matmul(kp[:], lhsT=wk_sb[:, ks, :],
                                             rhs=h1[:, ks, :],
                                             start=(ks == 0), stop=(ks == KS - 1))
                        rope_chunk(kp[:], j, g, kT[:, ds(NB * g, NB), j, :])
                        for m in range(NB):
                            vp = ps_b.tile([P, TOK], F32, tag="psb")
                            for ks in range(KS):
                                nc.tensor.matmul(
                                    vp[:, :HD], lhsT=h1[:, ks, ds(m * HB, HB)],
                                    rhs=wv_sb[:, ks, :],
                                    start=(ks == 0), stop=(ks == KS - 1))
                            nc.vector.tensor_copy(
                                vaug[:, (NB * g + m) * 2 + j, 1:], vp[:, :HD])

                # ---- attention (2 local heads, full 2048x2048, no mask) ----
                for hh in range(QH):
                    for qa in range(NQA):
                        att = attp.tile([P, NKC, TOKA], BF, tag="att")
                        for kc2 in range(NKC // 2):
                            sp = ps_a.tile([P, TOK], F32, tag="psa")
                            for half in range(2):
                                kc = 2 * kc2 + half
                                nc.tensor.matmul(sp[:, ds(half * TOKA, TOKA)],
                                                 lhsT=kT[:, kc // 2, kc % 2, :],
                                                 rhs=qT[:, hh, qa, :, :],
                                                 start=True, stop=True)
                            nc.scalar.activation(att[:, ds(2 * kc2, 2), :], sp[:],
                                                 AF.Exp, scale=SCALE)
                        for m in range(TOKA // P):
                            op_ = ps_b.tile([P, TOK], F32, tag="psb")
                            for kc in range(NKC):
                                nc.tensor.matmul(op_[:, :HD + 1],
                                                 lhsT=att[:, kc, ds(m * P, P)],
                                                 rhs=vaug[:, kc, :],
                                                 start=(kc == 0),
                                                 stop=(kc == NKC - 1))
                            rcp = scr.tile([P, 1], F32, tag="rcp")
                            nc.vector.reciprocal(rcp[:], op_[:, 0:1])
                            onrm = scr.tile([P, P], BF, tag="onrm")
                            nc.vector.tensor_scalar_mul(onrm[:], op_[:, 1:HD + 1],
                                                        rcp[:, :1])
                            tp = ps_c.tile([P, P], BF, tag="psc")
                            nc.tensor.transpose(tp[:], onrm[:], ident[:])
                            nc.vector.tensor_copy(
                                oT[:, hh, ds(qa * TOKA + m * P, P)], tp[:])

                # ---- o-proj (row-parallel) -> ReduceScatter -> residual ----
                aro_in = dram.tile([NCORES, 2, P, KS, HB], BF, tag=f"aroi{l}",
                                   name=f"aroi{l}")
                for qc in range(NQC):
                    t0 = qc * TOK
                    arst = retp.tile([P, KS, NB, HB], BF, tag="ret")
                    for hc in range(KS):
                        xo = ps_a.tile([P, NB, HB], F32, tag="psa")
                        for odc in range(ODC):
                            nc.tensor.matmul(xo[:], lhsT=wo_sb[:, odc, ds(hc * P, P)],
                                             rhs=oT[:, odc, ds(t0, TOK)],
                                             start=(odc == 0), stop=(odc == ODC - 1))
                        nc.vector.tensor_copy(arst[:, hc, :, :], xo[:])
                    # chunk qc holds ranks 2qc, 2qc+1, both halves
                    for b in range(2):
                        for j in range(2):
                            nc.sync.dma_start(aro_in[2 * qc + b, j],
                                              arst[:, :, 2 * b + j, :])
                rs_and_add(aro_in, f"o{l}")

                agx2 = gather_x(f"h2_{l}")

                # ---- MLP (column-parallel gate/up, row-parallel down) ----
                ard_in = dram.tile([NCORES, 2, P, KS, HB], BF, tag=f"ardi{l}",
                                   name=f"ardi{l}")
                for j in range(2):
                    for g in range(2):
                        h2 = hpool.tile([P, KS, TOK], BF, tag="hch")
                        load_chunk(agx2, j, g, h2)
                        rmsnorm_inplace(h2)
                        act = actp.tile([P, IB, TOK], BF, tag="act")
                        for ib in range(IB):
                            gp = ps_a.tile([P, TOK], F32, tag="psa")
                            for kh in range(2):
                                wg_sb = wstr.tile([P, KS // 2, HD], BF, tag="wg")
                                nc.sync.dma_start(
                                    wg_sb[:],
                                    wg_ap[l][:, ib, ds(kh * KS // 2, KS // 2)])
                                for k8 in range(KS // 2):
                                    ks = kh * (KS // 2) + k8
                                    nc.tensor.matmul(gp[:], lhsT=wg_sb[:, k8, :],
                                                     rhs=h2[:, ks, :],
                                                     start=(ks == 0),
                                                     stop=(ks == KS - 1))
                            up = ps_b.tile([P, TOK], F32, tag="psb")
                            for kh in range(2):
                                wu_sb = wstr.tile([P, KS // 2, HD], BF, tag="wu")
                                nc.sync.dma_start(
                                    wu_sb[:],
                                    wu_ap[l][:, ib, ds(kh * KS // 2, KS // 2)])
                                for k8 in range(KS // 2):
                                    ks = kh * (KS // 2) + k8
                                    nc.tensor.matmul(up[:], lhsT=wu_sb[:, k8, :],
                                                     rhs=h2[:, ks, :],
                                                     start=(ks == 0),
                                                     stop=(ks == KS - 1))
                            gs = scr.tile([P, TOK], BF, tag="xsq", bufs=3)
                            nc.scalar.activation(gs[:], gp[:], AF.Silu)
                            nc.vector.tensor_tensor(act[:, ib, :], gs[:], up[:],
                                                    OP.mult)
                        arst2 = retp.tile([P, KS, NB, HB], BF, tag="ret")
                        for gg in range(NHG):
                            wd_sb = wstr.tile([P, IB, WDC], BF, tag="wd")
                            nc.sync.dma_start(wd_sb[:], wd_ap[l][:, gg])
                            for hs in range(WDC // P):
                                hc = gg * (WDC // P) + hs
                                dx = ps_a.tile([P, NB, HB], F32, tag="psa")
                                for ib in range(IB):
                                    nc.tensor.matmul(
                                        dx[:], lhsT=wd_sb[:, ib, ds(hs * P, P)],
                                        rhs=act[:, ib, :],
                                        start=(ib == 0), stop=(ib == IB - 1))
                                nc.vector.tensor_copy(arst2[:, hc, :, :], dx[:])
                        for m in range(NB):
                            nc.sync.dma_start(ard_in[NB * g + m, j],
                                              arst2[:, :, m, :])
                rs_and_add(ard_in, f"d{l}")

            # ---- final rmsnorm on own tokens ----
            hf = own.tile([P, KS, TCC], BF, tag="hf")
            ssq = ps_c.tile([1, TOK], F32, tag="psc")
            for ks in range(KS):
                xsq = scr.tile([P, TOK], BF, tag="xsq", bufs=3)
                nc.vector.tensor_tensor(xsq[:, :TCC], x_own[:, ks, :],
                                        x_own[:, ks, :], OP.mult)
                nc.tensor.matmul(ssq[:, :TCC], lhsT=ones_h[:], rhs=xsq[:, :TCC],
                                 start=(ks == 0), stop=(ks == KS - 1))
            var = scr.tile([1, TOK], F32, tag="var", bufs=1)
            nc.vector.tensor_scalar(var[:, :TCC], ssq[:, :TCC], 1.0 / H, EPS,
                                    OP.mult, OP.add)
            rec = scr.tile([1, TOK], F32, tag="rec", bufs=1)
            nc.vector.reciprocal(rec[:, :TCC], var[:, :TCC])
            rstd = scr.tile([1, TOK], F32, tag="rstd", bufs=1)
            nc.scalar.activation(rstd[:, :TCC], rec[:, :TCC], AF.Sqrt)
            rb = ps_b.tile([P, TOK], F32, tag="psb")
            nc.tensor.matmul(rb[:, :TCC], lhsT=ones_r[:], rhs=rstd[:, :TCC],
                             start=True, stop=True)
            hf2 = hf
            for ks in range(KS):
                nc.vector.tensor_tensor(hf2[:, ks, :], x_own[:, ks, :],
                                        rb[:, :TCC], OP.mult)
            for ks in range(KS):
                fin = scr.tile([P, TCC], BF, tag="fin", bufs=1)
                nc.vector.tensor_scalar_mul(fin[:], hf2[:, ks, :],
                                            nw_sb[:, ds(ks, 1)])
                nc.sync.dma_start(out_ap[:, ks], fin[:])

    nc.compile()
    return nc


def _prep_inputs(input_ids, embed, Wq, Wk, Wv, Wo, Wg, Wu, Wd, ln1, ln2, norm_w):
    bf16 = ml_dtypes.bfloat16
    f32 = np.float32
    ids = np.asarray(input_ids).reshape(S)
    embed = np.asarray(embed, f32)
    ln1 = np.asarray(ln1, f32)
    ln2 = np.asarray(ln2, f32)

    # embedding gather on host; residual is H-major on device
    x0T = np.ascontiguousarray(embed[ids].T).reshape(KS, P, S)   # [KS, P, S] f32

    Wq = np.asarray(Wq, f32) * ln1[:, :, None]
    Wk = np.asarray(Wk, f32) * ln1[:, :, None]
    Wv = np.asarray(Wv, f32) * ln1[:, :, None]
    Wo = np.asarray(Wo, f32)
    Wg = np.asarray(Wg, f32) * ln2[:, :, None]
    Wu = np.asarray(Wu, f32) * ln2[:, :, None]
    Wd = np.asarray(Wd, f32)

    inv = 1.0 / (THETA ** (np.arange(0, HD, 2, dtype=np.float64) / HD))  # [64]
    fr = np.arange(S, dtype=np.float64)[None, :] * inv[:, None]          # [64, S]
    def chunk_order(t):
        # [P, S] -> [P, half j, group g, subblock m, 128] matching rope chunks
        return np.ascontiguousarray(
            t.reshape(P, 2, NB, 2, HB).transpose(0, 3, 1, 2, 4))
    cosT = chunk_order(np.concatenate([np.cos(fr), np.cos(fr)], 0)
                       ).astype(bf16).reshape(P, S)
    sinT = chunk_order(np.concatenate([np.sin(fr), np.sin(fr)], 0)
                       ).astype(bf16).reshape(P, S)

    rotT = np.zeros((P, P), f32)
    for m in range(HD // 2):
        rotT[m + HD // 2, m] = -1.0
    for m in range(HD // 2, HD):
        rotT[m - HD // 2, m] = 1.0
    rotT = rotT.astype(bf16)

    nwT = np.ascontiguousarray(np.asarray(norm_w, f32).reshape(KS, P).T)  # [P, KS]

    def colshard(w, lo, hi):
        # [L, H, N] cols [lo:hi) -> [L, P, KS, hi-lo]
        return np.ascontiguousarray(
            w[:, :, lo:hi].reshape(L, KS, P, hi - lo).transpose(0, 2, 1, 3)
        ).astype(bf16)

    in_maps = []
    for c in range(NCORES):
        wq = colshard(Wq, c * QH * HD, (c + 1) * QH * HD)
        wk = colshard(Wk, c * HD, (c + 1) * HD)
        wv = colshard(Wv, c * HD, (c + 1) * HD)
        # wo rows for this core's heads: [L, 256, H] -> [L, P, ODC, H]
        wo = np.ascontiguousarray(
            Wo[:, c * QH * HD:(c + 1) * QH * HD, :]
            .reshape(L, ODC, P, H).transpose(0, 2, 1, 3)).astype(bf16)
        # wg/wu cols for this core: [L, H, IC] -> [L, P, IB, KS, HD]
        wg = np.ascontiguousarray(
            Wg[:, :, c * IC:(c + 1) * IC]
            .reshape(L, KS, P, IB, HD).transpose(0, 2, 3, 1, 4)).astype(bf16)
        wu = np.ascontiguousarray(
            Wu[:, :, c * IC:(c + 1) * IC]
            .reshape(L, KS, P, IB, HD).transpose(0, 2, 3, 1, 4)).astype(bf16)
        # wd rows for this core: [L, IC, H] -> [L, P, NHG, IB, WDC]
        wd = np.ascontiguousarray(
            Wd[:, c * IC:(c + 1) * IC, :]
            .reshape(L, IB, P, NHG, WDC).transpose(0, 2, 3, 1, 4)).astype(bf16)
        x0 = np.ascontiguousarray(
            x0T[:, :, c * TCC:(c + 1) * TCC].transpose(1, 0, 2).astype(bf16))
        in_maps.append(dict(
            x0=x0, wq=wq, wk=wk, wv=wv, wo=wo, wg=wg, wu=wu, wd=wd,
            cosT=cosT, sinT=sinT, rotT=rotT, nwT=nwT,
        ))
    return in_maps


def _assemble(results):
    outT = np.empty((P, KS, S), np.float32)
    for c in range(NCORES):
        outT[:, :, c * TCC:(c + 1) * TCC] = np.asarray(
            results[c]["out"]).astype(np.float32)
    # [P, KS, S] -> [H, S] -> [S, H]
    return outT.transpose(1, 0, 2).reshape(H, S).T.copy().reshape(B, S, H)


def kernel(**inputs):
    global LAST_RESULT, LAST_NC, LAST_IN_MAPS
    in_maps = _prep_inputs(**inputs)
    nc = _build()
    res = bass_utils.run_bass_kernel_spmd(nc, in_maps, core_ids=list(range(NCORES)))
    LAST_RESULT = res
    LAST_NC = nc
    LAST_IN_MAPS = in_maps
    return _assemble(res.results)
